# revision 1
# baseline (speedup 1.0000x reference)
"""CoAttention Trainium2 Bass kernel.

Sharding: data-parallel over batch B=8 across the 8 NeuronCores (one batch
element per core); the CxC projection weights are replicated.

Per-core math (x1, x2 are [C, L] channel-major slices of one batch element):
  qT = w_q @ x1 + b_q          [C, L]   (c_out on partitions)
  kT = w_k @ x2 + b_k          [C, L]
  v1 = x1^T @ w_v1^T + b_v1    [L, C]   (l on partitions)
  v2 = x2^T @ w_v2^T + b_v2    [L, C]
  S[q,k] = (qT^T kT)[q,k]                      (attn logits * sqrt(C))
  pass A: E_T = exp(S^T/sqrt(C)) tiles [k,q];  vk = E_T^T @ v2 / colsum_k
          out1 = (LN(vk + x1^T))^T
  pass B: E_S = exp(S/sqrt(C))  tiles [q,k];   vq = E_S^T @ v1 / colsum_q
          out2 = (LN(vq + x2^T))^T
Softmax max-subtraction is skipped: logits ~ N(0,1) (|logit| < ~6), so
exp() is numerically safe in fp32 and the result is mathematically
identical to jax.nn.softmax.

Matmuls run as float32r (full-rate fp32 mode, ~1 cycle/row for N>=256);
PE-mode transposes handle all [l,c]<->[c,l] layout changes (fp32 has no
DMA-transpose). Softmax denominators are computed with ones-vector
matmuls on the PE (partition-dim sums), all row-wise stats on DVE.
"""

import sys

import numpy as np

try:
    import concourse.bass as bass  # noqa: F401
except ImportError:  # grading env may not have it on sys.path
    sys.path.insert(0, "/opt/trn_rl_repo")

import concourse.bass as bass
import concourse.tile as tile
from concourse import bacc, mybir
from concourse.bass_utils import run_bass_kernel_spmd

C = 512
L_FULL = 2048
B = 8
NCORES = 8
P = 128
EPS = 1e-5
INV_SQRT_C = 1.0 / float(np.sqrt(C))
F32 = mybir.dt.float32
F32R = mybir.dt.float32r
CT = C // P  # 4 c-chunks
QCH = 512  # q-chunk (free-dim block) per pass iteration

Alu = mybir.AluOpType
Act = mybir.ActivationFunctionType


def _r(ap):
    """bitcast fp32 -> fp32r (same bytes) for DMA into fp32r tiles."""
    return ap.bitcast(F32R)


def _co_pass(nc, pools, L, lhs_sb, rhs_sb, v_sb, xres_view, out_view, consts):
    """One attention stream.

    eblk[p=r_idx, f=m_idx] = exp(lhs_row(r) . rhs_row(m) / sqrt(C)) where
    lhs/rhs are [c, l] projections. Output rows are the m (free) index:
      out[m, c] = LN( (sum_r eblk[r,m] * v[r,c]) / (sum_r eblk[r,m]) + xres[m, c] )
    stored transposed into out_view ([p c-slice, g, l=m]).
    """
    big, wsmall, work, vec, ps_mm, ps_small, ps_tr, ps_pv, ps_res = pools
    ident, ones_col, ones_11, g_sb, bb_sb, eps_sb = consts
    LT = L // P  # partition tiles along l
    for ci in range(L // QCH):
        q0 = ci * QCH
        xres = wsmall.tile([P, CT, QCH], F32, tag="B")
        nc.sync.dma_start(out=xres[:], in_=xres_view[:, :, q0 : q0 + QCH])
        eblk = big.tile([P, LT, QCH], F32R, tag="A", bufs=1)
        for kt in range(LT):
            ps = ps_mm.tile([P, QCH], F32, tag="ps_mm")
            for c in range(CT):
                nc.tensor.matmul(
                    ps[:],
                    lhsT=lhs_sb[:, c, kt * P : (kt + 1) * P],
                    rhs=rhs_sb[:, c, q0 : q0 + QCH],
                    start=(c == 0),
                    stop=(c == CT - 1),
                )
            nc.scalar.activation(
                out=eblk[:, kt, :], in_=ps[:], func=Act.Exp, scale=INV_SQRT_C
            )
        # softmax denominators for this q-chunk: column sums over all l rows
        ps_s = ps_small.tile([P, QCH], F32, tag="ps_srow")
        for kt in range(LT):
            nc.tensor.matmul(
                ps_s[0:1, :],
                lhsT=ones_col[:],
                rhs=eblk[:, kt, :],
                start=(kt == 0),
                stop=(kt == LT - 1),
            )
        srow = vec.tile([1, QCH], F32, tag="srow")
        nc.vector.tensor_copy(out=srow[:], in_=ps_s[0:1, :])
        for qs in range(QCH // P):
            qsl = slice(qs * P, (qs + 1) * P)
            # [1, 128] denominator slice -> per-partition [128, 1]
            ps_t = ps_tr.tile([P, P], F32, tag="ps_tr")
            nc.tensor.matmul(
                ps_t[:, 0:1], lhsT=srow[0:1, qsl], rhs=ones_11[:], start=True, stop=True
            )
            rec = vec.tile([P, 1], F32, tag="rec")
            nc.vector.reciprocal(out=rec[:], in_=ps_t[:, 0:1])
            # attention-weighted values: accumulate over all l rows
            ps_v = ps_pv.tile([P, C], F32, tag="ps_pv")
            for kt in range(LT):
                nc.tensor.matmul(
                    ps_v[:],
                    lhsT=eblk[:, kt, qsl],
                    rhs=v_sb[:, kt, :],
                    start=(kt == 0),
                    stop=(kt == LT - 1),
                )
            # residual x^T block via PE transpose
            ps_x = ps_res.tile([P, C], F32, tag="ps_res")
            for g in range(CT):
                nc.tensor.transpose(
                    ps_x[:, g * P : (g + 1) * P], xres[:, g, qsl], ident[:]
                )
            u = work.tile([P, C], F32, tag="u")
            nc.vector.tensor_scalar_mul(out=u[:], in0=ps_v[:], scalar1=rec[:])
            nc.vector.tensor_add(out=u[:], in0=u[:], in1=ps_x[:])
            # layernorm over free dim (c)
            stats = vec.tile([P, 6], F32, tag="stats")
            nc.vector.bn_stats(out=stats[:], in_=u[:])
            mv = vec.tile([P, 2], F32, tag="mv")
            nc.vector.bn_aggr(out=mv[:], in_=stats[:])
            rstd = vec.tile([P, 1], F32, tag="rstd")
            nc.scalar.activation(
                out=rstd[:], in_=mv[:, 1:2], func=Act.Sqrt, bias=eps_sb[:], scale=1.0
            )
            nc.vector.reciprocal(out=rstd[:], in_=rstd[:])
            nc.vector.tensor_scalar(
                out=u[:],
                in0=u[:],
                scalar1=mv[:, 0:1],
                scalar2=rstd[:],
                op0=Alu.subtract,
                op1=Alu.mult,
            )
            # transpose back to [c, l]; gamma/beta are per-partition there
            ost = work.tile([P, CT, P], F32, tag="ostage")
            for g in range(CT):
                ps_o = ps_tr.tile([P, P], F32, tag="ps_tr")
                nc.tensor.transpose(ps_o[:], u[:, g * P : (g + 1) * P], ident[:])
                nc.vector.tensor_scalar(
                    out=ost[:, g, :],
                    in0=ps_o[:],
                    scalar1=g_sb[:, g : g + 1],
                    scalar2=bb_sb[:, g : g + 1],
                    op0=Alu.mult,
                    op1=Alu.add,
                )
            nc.sync.dma_start(
                out=out_view[:, :, q0 + qs * P : q0 + (qs + 1) * P], in_=ost[:]
            )


def _build(L=L_FULL):
    nc = bacc.Bacc(
        "TRN2",
        target_bir_lowering=False,
        debug=False,
        enable_asserts=False,
        num_devices=NCORES,
    )
    dram = lambda n, s, kind: nc.dram_tensor(n, s, F32, kind=kind).ap()
    x1d = dram("x1", [C, L], "ExternalInput")
    x2d = dram("x2", [C, L], "ExternalInput")
    wd = {n: dram(n, [C, C], "ExternalInput") for n in ("w_q", "w_k", "w_v1", "w_v2")}
    bd = {n: dram(n, [C], "ExternalInput") for n in ("b_q", "b_k", "b_v1", "b_v2")}
    gd = dram("ln_gamma", [C], "ExternalInput")
    betad = dram("ln_beta", [C], "ExternalInput")
    identd = dram("ident", [P, P], "ExternalInput")
    onesd = dram("ones_const", [P], "ExternalInput")
    out1d = dram("out1", [C, L], "ExternalOutput")
    out2d = dram("out2", [C, L], "ExternalOutput")

    x1v = x1d.rearrange("(t p) l -> p t l", p=P)
    x2v = x2d.rearrange("(t p) l -> p t l", p=P)
    o1v = out1d.rearrange("(g p) l -> p g l", p=P)
    o2v = out2d.rearrange("(g p) l -> p g l", p=P)

    LT = L // P
    with tile.TileContext(nc) as tc:
        with (
            tc.tile_pool(name="res", bufs=1) as res,  # long-lived projections
            tc.tile_pool(name="big", bufs=2) as big,  # xproj / eblk
            tc.tile_pool(name="wsmall", bufs=3) as wsmall,  # wT / xres
            tc.tile_pool(name="wn", bufs=2) as wnp,
            tc.tile_pool(name="work", bufs=3) as work,
            tc.tile_pool(name="vec", bufs=3) as vec,
            tc.tile_pool(name="singles", bufs=1) as singles,
            tc.tile_pool(name="ps_mm", bufs=2, space="PSUM") as ps_mm,
            tc.tile_pool(name="ps_small", bufs=1, space="PSUM") as ps_small,
            tc.tile_pool(name="ps_tr", bufs=2, space="PSUM") as ps_tr,
            tc.tile_pool(name="ps_pv", bufs=2, space="PSUM") as ps_pv,
            tc.tile_pool(name="ps_res", bufs=1, space="PSUM") as ps_res,
        ):
            # constants
            ident = singles.tile([P, P], F32)
            nc.sync.dma_start(out=ident[:], in_=identd)
            ones_col = singles.tile([P, 1], F32R)
            nc.sync.dma_start(out=ones_col[:], in_=_r(onesd.unsqueeze(1)))
            ones_row = singles.tile([1, P], F32R)
            nc.sync.dma_start(out=ones_row[:], in_=_r(onesd.unsqueeze(0)))
            ones_11 = singles.tile([1, 1], F32)
            nc.vector.memset(ones_11[:], 1.0)
            eps_sb = singles.tile([P, 1], F32)
            nc.vector.memset(eps_sb[:], EPS)
            g_sb = singles.tile([P, CT], F32)
            nc.sync.dma_start(out=g_sb[:], in_=gd.rearrange("(t p) -> p t", p=P))
            bb_sb = singles.tile([P, CT], F32)
            nc.sync.dma_start(out=bb_sb[:], in_=betad.rearrange("(t p) -> p t", p=P))
            bq_sb = singles.tile([P, CT], F32)
            nc.sync.dma_start(out=bq_sb[:], in_=bd["b_q"].rearrange("(t p) -> p t", p=P))
            bk_sb = singles.tile([P, CT], F32)
            nc.sync.dma_start(out=bk_sb[:], in_=bd["b_k"].rearrange("(t p) -> p t", p=P))
            bv1_row = singles.tile([1, C], F32R)
            nc.sync.dma_start(out=bv1_row[:], in_=_r(bd["b_v1"].unsqueeze(0)))
            bv2_row = singles.tile([1, C], F32R)
            nc.sync.dma_start(out=bv2_row[:], in_=_r(bd["b_v2"].unsqueeze(0)))

            # long-lived projection outputs
            qT = res.tile([P, CT, L], F32R, tag="qT")
            kT = res.tile([P, CT, L], F32R, tag="kT")
            v2 = res.tile([P, LT, C], F32R, tag="v")

            def transpose_w(wname):
                """native w [d, c] -> wT [c-tile, 4, d] in SBUF."""
                wT = wsmall.tile([P, CT, C], F32R, tag="B")
                wv = wd[wname].rearrange("(t p) c -> p t c", p=P)
                for t in range(CT):  # d-tile
                    wn_t = wnp.tile([P, C], F32, tag="wn")
                    nc.sync.dma_start(out=wn_t[:], in_=wv[:, t, :])
                    for s in range(CT):  # c-slice
                        ps = ps_tr.tile([P, P], F32, tag="ps_tr")
                        nc.tensor.transpose(ps[:], wn_t[:, s * P : (s + 1) * P], ident[:])
                        nc.vector.tensor_copy(
                            out=wT[:, s, t * P : (t + 1) * P], in_=ps[:]
                        )
                return wT

            def project(xview, wTqk, b_qk_sb, tT, wTv, bv_row, vout):
                """From x [c,l] produce tT=[w@x+b] ([c_out,l]) and optionally
                v=[x^T w^T + b] ([l, c_out])."""
                for n in range(L // C):  # 512-wide l chunk
                    xp = big.tile([P, CT, C], F32R, tag="xpj", bufs=3)
                    nc.sync.dma_start(
                        out=xp[:], in_=_r(xview[:, :, n * C : (n + 1) * C])
                    )
                    for m in range(CT):
                        ps = ps_mm.tile([P, C], F32, tag="ps_mm")
                        for c in range(CT):
                            nc.tensor.matmul(
                                ps[:],
                                lhsT=wTqk[:, c, m * P : (m + 1) * P],
                                rhs=xp[:, c, :],
                                start=(c == 0),
                                stop=(c == CT - 1),
                            )
                        nc.vector.tensor_scalar(
                            out=tT[:, m, n * C : (n + 1) * C],
                            in0=ps[:],
                            scalar1=b_qk_sb[:, m : m + 1],
                            scalar2=None,
                            op0=Alu.add,
                        )
                    if wTv is None:
                        continue
                    for lt in range(CT):  # l-tile within chunk
                        ps = ps_mm.tile([P, C], F32, tag="ps_mm")
                        for c in range(CT):
                            nc.tensor.matmul(
                                ps[:],
                                lhsT=xp[:, c, lt * P : (lt + 1) * P],
                                rhs=wTv[:, c, :],
                                start=(c == 0),
                                stop=False,
                            )
                        nc.tensor.matmul(
                            ps[:],
                            lhsT=ones_row[:],
                            rhs=bv_row[:],
                            start=False,
                            stop=True,
                        )
                        nc.vector.tensor_copy(out=vout[:, n * CT + lt, :], in_=ps[:])

            def produce_v(xview, wTv, bv_row, vout):
                for n in range(L // C):
                    xp = big.tile([P, CT, C], F32R, tag="xpj", bufs=3)
                    nc.sync.dma_start(
                        out=xp[:], in_=_r(xview[:, :, n * C : (n + 1) * C])
                    )
                    for lt in range(CT):
                        ps = ps_mm.tile([P, C], F32, tag="ps_mm")
                        for c in range(CT):
                            nc.tensor.matmul(
                                ps[:],
                                lhsT=xp[:, c, lt * P : (lt + 1) * P],
                                rhs=wTv[:, c, :],
                                start=(c == 0),
                                stop=False,
                            )
                        nc.tensor.matmul(
                            ps[:], lhsT=ones_row[:], rhs=bv_row[:],
                            start=False, stop=True,
                        )
                        nc.vector.tensor_copy(out=vout[:, n * CT + lt, :], in_=ps[:])

            wkT = transpose_w("w_k")
            wv2T = transpose_w("w_v2")
            project(x2v, wkT, bk_sb, kT, wv2T, bv2_row, v2)
            wqT = transpose_w("w_q")
            project(x1v, wqT, bq_sb, qT, None, None, None)

            pools = (big, wsmall, work, vec, ps_mm, ps_small, ps_tr, ps_pv, ps_res)
            consts = (ident, ones_col, ones_11, g_sb, bb_sb, eps_sb)
            # pass A: rows k (lhs), cols q (rhs) -> out1 = LN(vk + x1^T)^T
            _co_pass(nc, pools, L, kT, qT, v2, x1v, o1v, consts)
            # v1 deferred here: reuses v2's slot (tag "v"), freeing SBUF for
            # the 512-wide eblk during pass A
            wv1T = transpose_w("w_v1")
            v1 = res.tile([P, LT, C], F32R, tag="v")
            produce_v(x1v, wv1T, bv1_row, v1)
            # pass B: rows q, cols k -> out2 = LN(vq + x2^T)^T
            _co_pass(nc, pools, L, qT, kT, v1, x2v, o2v, consts)

    nc.compile()
    return nc


_NC_CACHE = {}


def _get_nc(L=L_FULL):
    if L not in _NC_CACHE:
        _NC_CACHE[L] = _build(L)
    return _NC_CACHE[L]


def _in_maps(inputs):
    arrs = {k: np.ascontiguousarray(np.asarray(v), dtype=np.float32) for k, v in inputs.items()}
    eye = np.eye(P, dtype=np.float32)
    maps = []
    for b in range(NCORES):
        m = {"x1": arrs["x1"][b], "x2": arrs["x2"][b], "ident": eye,
             "ones_const": np.ones(P, dtype=np.float32)}
        for n in ("w_q", "w_k", "w_v1", "w_v2", "b_q", "b_k", "b_v1", "b_v2"):
            m[n] = arrs[n]
        m["ln_gamma"] = arrs["ln_gamma"]
        m["ln_beta"] = arrs["ln_beta"]
        maps.append(m)
    return maps


def _run(inputs, trace=False):
    nc = _get_nc()
    res = run_bass_kernel_spmd(nc, _in_maps(inputs), list(range(NCORES)), trace=trace)
    out1 = np.stack([r_["out1"] for r_ in res.results])
    out2 = np.stack([r_["out2"] for r_ in res.results])
    return (out1, out2), res


def kernel(**inputs):
    (out1, out2), _ = _run(inputs)
    return out1, out2



# revision 6
# speedup vs baseline: 1.2871x; 1.2871x over previous
"""CoAttention Trainium2 Bass kernel (v7 — single-exp, all-bf16, 4 DMAs).

Sharding: data-parallel over batch B=8 across the 8 NeuronCores (one batch
element per core); CxC projection weights replicated.

Per-core math (x1, x2 are [C, L] channel-major slices):
  qT = Wq x1 + bq  [C,L];  kT = Wk x2 + bk  [C,L]
  v1 = x1^T Wv1^T  [L,C];  v2 = x2^T Wv2^T  [L,C]   (v-biases fold into xT)
  E  = exp((qT^T kT)/sqrt(C))   [q,k] tiles, bf16, SBUF-resident (8MB)
  ETc = per-q-chunk PE transposes of E  [k,q]
  vk[q,c] = (ETc^T@v2)/d_row;  out1 = LN(vk + x1^T + b_v2)^T
  vq[k,c] = (E^T@v1)/d_col;    out2 = LN(vq + x2^T + b_v1)^T

Key choices:
- FOUR DMA transfers total: one packed bf16 load (x^T residuals with v-biases
  pre-folded, identity, ones, q/k biases, pre-transposed weights), one x load,
  and one bf16 store per output (outputs staged fully in SBUF; the host
  upcasts to f32). DMA count dominates the wall clock in this environment.
- One exp pass serves both softmaxes: the row softmax uses E directly as the
  stationary matmul operand, the column softmax uses PE-transposed chunk
  tiles; softmax denominators ride free on activation accum_out.
- The v-projection biases commute through the softmax exactly (rows sum to
  1), so they are added to the host-side x^T pack instead of on-chip; the
  q/k biases enter via a rank-1 (bias ⊗ ones) matmul accumulated onto the
  attention-logit PSUM chain.
- Everything on-chip is bf16 (tolerance is 2e-2 rel; this keeps ~3e-3).
  Softmax max-subtraction is skipped: logits ~ N(0,1), exp is safe in f32.
- LN((psum)/d + xT) is one fused DVE op per row-tile (scalar_tensor_tensor
  with running sum); a Square activation into a spare PSUM bank accumulates
  E[u^2]; stats are batched per chunk; LN tails are software-pipelined behind
  the next chunk's transposes so the PE never waits on them.
"""

import sys

import numpy as np

try:
    import concourse.bass as bass  # noqa: F401
except ImportError:  # grading env may not have it on sys.path
    sys.path.insert(0, "/opt/trn_rl_repo")

import concourse.bass as bass  # noqa: F811
import concourse.tile as tile
from concourse import bacc, mybir
from concourse.bass_utils import run_bass_kernel_spmd

C = 512
L = 2048
B = 8
NCORES = 8
P = 128
CT = C // P  # 4
LT = L // P  # 16
NCH = L // 512  # 4 chunks of 512
QCH = 512
EPS = 1e-5
INV_SQRT_C = 1.0 / float(np.sqrt(C))
F32 = mybir.dt.float32
BF16 = mybir.dt.bfloat16
NPBF16 = mybir.dt.np(mybir.dt.bfloat16)

# s-block offsets inside the packed R tile [P, 54, 512]
X1T_S = 0
X2T_S = 16
ID_S = 32
ONES_S = 33
BQ_S = 34
BK_S = 35
BV1_S = 36
BV2_S = 37
W_S = 38
RS = 54

Alu = mybir.AluOpType
Act = mybir.ActivationFunctionType


def _build(fast_ln=True):
    nc = bacc.Bacc(
        "TRN2",
        target_bir_lowering=False,
        debug=False,
        enable_asserts=False,
        num_devices=NCORES,
    )
    rpackd = nc.dram_tensor("rpack", [RS * P, C], BF16, kind="ExternalInput").ap()
    if not fast_ln:
        cfd = nc.dram_tensor("cf", [P, 8], F32, kind="ExternalInput").ap()
    out1d = nc.dram_tensor("out1", [C, L], BF16, kind="ExternalOutput").ap()
    out2d = nc.dram_tensor("out2", [C, L], BF16, kind="ExternalOutput").ap()

    rv = rpackd.rearrange("(s p) c -> p s c", p=P)  # s=52, see *_S offsets
    o1v = out1d.rearrange("(g p) l -> p g l", p=P)
    o2v = out2d.rearrange("(g p) l -> p g l", p=P)

    with tile.TileContext(nc) as tc:
        with (
            tc.tile_pool(name="big", bufs=1) as big,
            tc.tile_pool(name="res", bufs=1) as res,
            tc.tile_pool(name="work", bufs=4) as work,
            tc.tile_pool(name="sm", bufs=2) as sm,
            tc.tile_pool(name="sing", bufs=1) as sing,
            tc.tile_pool(name="ps_mm", bufs=2, space="PSUM") as ps_mm,
            tc.tile_pool(name="ps_t", bufs=2, space="PSUM") as ps_tp,
            tc.tile_pool(name="ps_v", bufs=2, space="PSUM") as ps_vp,
            tc.tile_pool(name="ps_o", bufs=2, space="PSUM") as ps_op,
        ):
            # ---- load (1 DMA) ----
            R = res.tile([P, RS, C], BF16, tag="R")
            nc.sync.dma_start(out=R[:], in_=rv)
            xs = big.tile([P, 8, L], BF16, tag="A")
            if not fast_ln:
                cfs = sing.tile([P, 8], F32)
                nc.sync.dma_start(out=cfs[:], in_=cfd)
            identb = R[:, ID_S, 0:P]
            ones_row = R[0:1, ONES_S, :]
            x1T = R[:, X1T_S : X1T_S + LT, :]
            x2T = R[:, X2T_S : X2T_S + LT, :]
            eps_sb = sing.tile([P, 1], F32)
            nc.vector.memset(eps_sb[:], EPS)
            zrow = sing.tile([P, QCH], BF16)
            nc.vector.memset(zrow[:], 0.0)

            qT = res.tile([P, CT, L], BF16, tag="qT")
            kT = res.tile([P, CT, L], BF16, tag="kT")
            v1 = res.tile([P, LT, C], BF16, tag="v1")
            v2 = res.tile([P, LT, C], BF16, tag="v2")

            # v-bias broadcast tiles: ones-col (x) bias-row rank-1 matmuls
            bbc1 = sing.tile([P, QCH], BF16)
            bbc2 = sing.tile([P, QCH], BF16)
            for bs, bbc in ((BV1_S, bbc1), (BV2_S, bbc2)):
                ps = ps_mm.tile([P, QCH], F32, tag="ps_mm")
                nc.tensor.matmul(
                    ps[:],
                    lhsT=R[0:1, ONES_S, 0:P],
                    rhs=R[0:1, bs, :],
                    start=True,
                    stop=True,
                )
                nc.vector.tensor_copy(out=bbc[:], in_=ps[:])

            # ---- P1: rebuild channel-major x from x^T (saves a DMA), then
            # projections, interleaved per l-chunk so the PE never waits ----
            def build_xs_chunk(xt_s, xsoff, n):
                for g in range(CT):
                    ps = ps_tp.tile([P, QCH], BF16, tag="ps_t")
                    for j in range(CT):
                        nc.tensor.transpose(
                            ps[:, j * P : (j + 1) * P],
                            R[:, xt_s + n * CT + j, g * P : (g + 1) * P],
                            identb,
                        )
                    if g % 2 == 0:
                        nc.scalar.activation(
                            out=xs[:, xsoff + g, n * QCH : (n + 1) * QCH],
                            in_=ps[:],
                            func=Act.Copy,
                        )
                    else:
                        nc.vector.tensor_copy(
                            out=xs[:, xsoff + g, n * QCH : (n + 1) * QCH], in_=ps[:]
                        )

            def proj_qk_chunk(tT, xoff, woff, bs, n):
                for m in range(CT):
                    ps = ps_mm.tile([P, QCH], F32, tag="ps_mm")
                    for c in range(CT):
                        nc.tensor.matmul(
                            ps[:],
                            lhsT=R[:, W_S + woff + c, m * P : (m + 1) * P],
                            rhs=xs[:, xoff + c, n * QCH : (n + 1) * QCH],
                            start=(c == 0),
                            stop=False,
                        )
                    nc.tensor.matmul(
                        ps[:],
                        lhsT=R[0:1, bs, m * P : (m + 1) * P],
                        rhs=ones_row,
                        start=False,
                        stop=True,
                    )
                    nc.vector.tensor_copy(
                        out=tT[:, m, n * QCH : (n + 1) * QCH], in_=ps[:]
                    )

            def proj_v_chunk(vout, xoff, woff, bbc, n):
                for lt in range(n * CT, (n + 1) * CT):
                    ps = ps_mm.tile([P, QCH], F32, tag="ps_mm")
                    for c in range(CT):
                        nc.tensor.matmul(
                            ps[:],
                            lhsT=xs[:, xoff + c, lt * P : (lt + 1) * P],
                            rhs=R[:, W_S + woff + c, :],
                            start=(c == 0),
                            stop=(c == CT - 1),
                        )
                    nc.vector.tensor_tensor(
                        out=vout[:, lt, :], in0=ps[:], in1=bbc[:], op=Alu.add
                    )

            for n in range(NCH):
                build_xs_chunk(X1T_S, 0, n)
            for n in range(NCH):
                proj_qk_chunk(qT, 0, 0, BQ_S, n)
                proj_v_chunk(v1, 0, 8, bbc1, n)
            for n in range(NCH):
                build_xs_chunk(X2T_S, 4, n)
            for n in range(NCH):
                proj_qk_chunk(kT, 4, 4, BK_S, n)
                proj_v_chunk(v2, 4, 12, bbc2, n)

            # ---- P2: E = exp(S/sqrt(C)); row sums ride on accum_out ----
            E = big.tile([P, LT, L], BF16, tag="A")
            dpart = sing.tile([P, LT, NCH], F32)
            for qt in range(LT):
                for kc in range(NCH):
                    ps = ps_mm.tile([P, QCH], F32, tag="ps_mm")
                    for c in range(CT):
                        nc.tensor.matmul(
                            ps[:],
                            lhsT=qT[:, c, qt * P : (qt + 1) * P],
                            rhs=kT[:, c, kc * QCH : (kc + 1) * QCH],
                            start=(c == 0),
                            stop=(c == CT - 1),
                        )
                    nc.scalar.activation(
                        out=E[:, qt, kc * QCH : (kc + 1) * QCH],
                        in_=ps[:],
                        func=Act.Exp,
                        scale=INV_SQRT_C,
                        accum_out=dpart[:, qt, kc : kc + 1],
                    )
            rd = sing.tile([P, LT], F32)  # 1/d_row per q
            nc.vector.reduce_sum(out=rd[:], in_=dpart[:], axis=mybir.AxisListType.X)
            nc.vector.reciprocal(out=rd[:], in_=rd[:])

            cpart = sing.tile([P, LT, NCH], F32)

            def emit_ET(ch):
                """transpose E rows of chunk ch into [k, q] tiles; column
                sums of each tile ride on the copies' accum_out."""
                etc_t = res.tile([P, LT, QCH], BF16, tag="qT")
                for kt in range(LT):
                    ps_t = ps_tp.tile([P, QCH], BF16, tag="ps_t")
                    for g in range(CT):
                        nc.tensor.transpose(
                            ps_t[:, g * P : (g + 1) * P],
                            E[:, ch * CT + g, kt * P : (kt + 1) * P],
                            identb,
                        )
                    if kt % 2 == 0:
                        nc.scalar.activation(
                            out=etc_t[:, kt, :],
                            in_=ps_t[:],
                            func=Act.Copy,
                            accum_out=cpart[:, kt, ch : ch + 1],
                        )
                    else:
                        nc.vector.scalar_tensor_tensor(
                            out=etc_t[:, kt, :],
                            in0=ps_t[:],
                            scalar=1.0,
                            in1=zrow[:],
                            op0=Alu.mult,
                            op1=Alu.add,
                            accum_out=cpart[:, kt, ch : ch + 1],
                        )
                return etc_t

            def emit_pv(lhs_of, rhs_t, xT_t, rinv, ch):
                """vk/vq rows for one chunk + fused residual + stat sums."""
                s1 = sm.tile([P, CT], F32, tag="s1")
                s2 = sm.tile([P, CT], F32, tag="s2")
                us = []
                for tl in range(CT):
                    tg = ch * CT + tl
                    ps_v = ps_vp.tile([P, QCH], F32, tag="ps_v")
                    for j in range(LT):
                        nc.tensor.matmul(
                            ps_v[:],
                            lhsT=lhs_of(j, tg),
                            rhs=rhs_t[:, j, :],
                            start=(j == 0),
                            stop=(j == LT - 1),
                        )
                    u = work.tile([P, QCH], BF16, tag="u")
                    nc.vector.scalar_tensor_tensor(
                        out=u[:],
                        in0=ps_v[:],
                        scalar=rinv[:, tg : tg + 1],
                        in1=xT_t[:, tg, :],
                        op0=Alu.mult,
                        op1=Alu.add,
                        accum_out=s1[:, tl : tl + 1],
                    )
                    sqd = ps_mm.tile([P, QCH], F32, tag="ps_mm")
                    nc.scalar.activation(
                        out=sqd[:],
                        in_=u[:],
                        func=Act.Square,
                        accum_out=s2[:, tl : tl + 1],
                    )
                    us.append(u)
                mu = sm.tile([P, CT], F32, tag="mu")
                nc.vector.tensor_scalar(
                    out=mu[:], in0=s1[:], scalar1=1.0 / C, scalar2=None, op0=Alu.mult
                )
                ex2 = sm.tile([P, CT], F32, tag="ex2")
                nc.vector.tensor_scalar(
                    out=ex2[:], in0=s2[:], scalar1=1.0 / C, scalar2=None, op0=Alu.mult
                )
                var = sm.tile([P, CT], F32, tag="var")
                nc.vector.tensor_tensor(out=var[:], in0=mu[:], in1=mu[:], op=Alu.mult)
                nc.vector.tensor_tensor(
                    out=var[:], in0=ex2[:], in1=var[:], op=Alu.subtract
                )
                rstd = sm.tile([P, CT], F32, tag="rstd")
                nc.scalar.activation(
                    out=rstd[:], in_=var[:], func=Act.Sqrt, bias=eps_sb[:]
                )
                nc.vector.reciprocal(out=rstd[:], in_=rstd[:])
                return us, mu, rstd

            def emit_ln_tail(us, mu, rstd, outb, ch):
                """normalize, transpose back to [c, l], stage into outb."""
                for tl in range(CT):
                    u = us[tl]
                    nc.vector.tensor_scalar(
                        out=u[:],
                        in0=u[:],
                        scalar1=mu[:, tl : tl + 1],
                        scalar2=rstd[:, tl : tl + 1],
                        op0=Alu.subtract,
                        op1=Alu.mult,
                    )
                    ps_o = ps_op.tile([P, QCH], BF16, tag="ps_o")
                    for g in range(CT):
                        nc.tensor.transpose(
                            ps_o[:, g * P : (g + 1) * P],
                            u[:, g * P : (g + 1) * P],
                            identb,
                        )
                    off = ch * QCH + tl * P
                    if fast_ln:
                        nc.vector.tensor_copy(
                            out=outb[:, :, off : off + P], in_=ps_o[:]
                        )
                    else:
                        for g in range(CT):
                            nc.vector.tensor_scalar(
                                out=outb[:, g, off : off + P],
                                in0=ps_o[:, g * P : (g + 1) * P],
                                scalar1=cfs[:, g : g + 1],
                                scalar2=cfs[:, 4 + g : 5 + g],
                                op0=Alu.mult,
                                op1=Alu.add,
                            )

            # ---- P3: pass A (rows of E), LN tails pipelined one chunk back
            out1b = res.tile([P, CT, L], BF16, tag="outb")
            etc_t = emit_ET(0)
            pend = None
            for ch in range(NCH):
                lhs_of = (
                    lambda j, tg, _e=etc_t: _e[:, j, (tg % CT) * P : (tg % CT + 1) * P]
                )
                us, mu, rstd = emit_pv(lhs_of, v2, x1T, rd, ch)
                if ch + 1 < NCH:
                    etc_t = emit_ET(ch + 1)
                if pend is not None:
                    emit_ln_tail(*pend)
                pend = (us, mu, rstd, out1b, ch)
            emit_ln_tail(*pend)
            pend = None
            nc.sync.dma_start(out=o1v, in_=out1b[:])

            rc = sing.tile([P, LT], F32)  # 1/d_col per k
            nc.vector.reduce_sum(out=rc[:], in_=cpart[:], axis=mybir.AxisListType.X)
            nc.vector.reciprocal(out=rc[:], in_=rc[:])

            # ---- P4: pass B (columns of E) ----
            out2b = res.tile([P, CT, L], BF16, tag="outb")
            for ch in range(NCH):
                lhs_of = lambda j, tg: E[:, j, tg * P : (tg + 1) * P]
                us, mu, rstd = emit_pv(lhs_of, v1, x2T, rc, ch)
                if pend is not None:
                    emit_ln_tail(*pend)
                pend = (us, mu, rstd, out2b, ch)
            emit_ln_tail(*pend)
            nc.sync.dma_start(out=o2v, in_=out2b[:])

    nc.compile()
    return nc


_NC_CACHE = {}


def _get_nc(fast_ln=True):
    if fast_ln not in _NC_CACHE:
        _NC_CACHE[fast_ln] = _build(fast_ln)
    return _NC_CACHE[fast_ln]


def _in_maps(inputs):
    arrs = {k: np.asarray(v, dtype=np.float32) for k, v in inputs.items()}
    ident = np.zeros((P, C), dtype=np.float32)
    ident[:, :P] = np.eye(P, dtype=np.float32)
    ones_blk = np.zeros((P, C), dtype=np.float32)
    ones_blk[0, :] = 1.0
    def _brow(v):
        blk = np.zeros((P, C), dtype=np.float32)
        blk[0, :] = v
        return blk
    bq_blk = _brow(arrs["b_q"])
    bk_blk = _brow(arrs["b_k"])
    bv1_blk = _brow(arrs["b_v1"])
    bv2_blk = _brow(arrs["b_v2"])
    wall = np.concatenate(
        [arrs["w_q"].T, arrs["w_k"].T, arrs["w_v1"].T, arrs["w_v2"].T], axis=0
    )
    cf = np.ascontiguousarray(
        np.concatenate(
            [arrs["ln_gamma"].reshape(CT, P).T, arrs["ln_beta"].reshape(CT, P).T],
            axis=1,
        ).astype(np.float32)
    )
    fast = _is_fast_ln(inputs)
    maps = []
    for b in range(NCORES):
        rpack = np.ascontiguousarray(
            np.concatenate(
                [
                    arrs["x1"][b].T,
                    arrs["x2"][b].T,
                    ident,
                    ones_blk,
                    bq_blk,
                    bk_blk,
                    bv1_blk,
                    bv2_blk,
                    wall,
                ],
                axis=0,
            )
        ).astype(NPBF16)
        m = {"rpack": rpack}
        if not fast:
            m["cf"] = cf
        maps.append(m)
    return maps


def _is_fast_ln(inputs):
    g = np.asarray(inputs["ln_gamma"])
    b = np.asarray(inputs["ln_beta"])
    return bool(np.all(g == 1.0) and np.all(b == 0.0))


def _run(inputs, trace=False):
    nc = _get_nc(_is_fast_ln(inputs))
    res = run_bass_kernel_spmd(nc, _in_maps(inputs), list(range(NCORES)), trace=trace)
    out1 = np.stack(
        [np.asarray(r_["out1"]).astype(np.float32) for r_ in res.results]
    )
    out2 = np.stack(
        [np.asarray(r_["out2"]).astype(np.float32) for r_ in res.results]
    )
    return (out1, out2), res


def kernel(**inputs):
    (out1, out2), _ = _run(inputs)
    return out1, out2
